# revision 5
# baseline (speedup 1.0000x reference)
"""HardAttentionLayer Trainium2 kernel, v2.

Math (forward value only):
  pos_emb = x + pe
  Ksum[b] = (xsum[b] + pesum) @ Wk.T * scale      (xsum[b] = sum_s x[b,s])
  v[b,n]  = Wq_n.T @ Ksum[b, nA:(n+1)A]
  y[b,n,s] = (x[b,s] + pe[s]) . v[b,n] + gumbel
  out[b,n] = x[b, argmax_s y]

v2 structure (vs v1): 2 super-groups of 32 batches per core; x transposed
via PE (25x8 128x128 tmode matmuls per sg) into one qbuf; xsum via DVE
segmented reduces over qbuf; Ksum via fat matmuls (weights moving, ap=512);
v via per-head 128x128 stationary matmuls; logits via fat matmuls
(v stationary [h,128bn], qbuf moving ap=400) computing a 128x400 block of
which the block-diagonal [8,100] strips are extracted during the
gumbel-add; pe contribution via extra qbuf columns holding pe.T.

Sharding: pure data parallel over batch, 64 batches per core on 8 cores.
"""

import math
from contextlib import ExitStack

import numpy as np

import concourse.bass as bass
import concourse.tile as tile
from concourse import bacc, mybir
from concourse.bass_utils import run_bass_kernel_spmd
from concourse.masks import make_identity

F32 = mybir.dt.float32
U32 = mybir.dt.uint32

B, S, H = 512, 100, 1024
A, N = 128, 8
NCORES = 8
BC = B // NCORES          # 64 batches per core
SG = 2                    # super-groups per core
GB = BC // SG             # 32 batches per super-group
ROWS = GB * S             # 3200 rows per super-group
NT = ROWS // 128          # 25 full 128-row tiles
QW = ROWS                 # qbuf cols
SCALE = 1.0 / (math.sqrt(H) * S)

# matmul-mode flags (validated on hardware via micro benches)
TMODE_V = False           # is_transpose path for v projection
TMODE_LG = False          # is_transpose path for logits GEMM
TMODE_KS = False          # is_transpose path for Ksum fat matmuls

_NC_CACHE = {}
LAST_RESULT = None


def _build_nc():
    nc = bacc.Bacc("TRN2", target_bir_lowering=False, debug=False)

    x = nc.dram_tensor("x", [BC * S, H], F32, kind="ExternalInput").ap()
    wkt = nc.dram_tensor("wkt", [128, 8, H], F32, kind="ExternalInput").ap()
    wq = nc.dram_tensor("wq", [128, 8, H], F32, kind="ExternalInput").ap()
    pet = nc.dram_tensor("pet", [128, 8, S], F32, kind="ExternalInput").ap()
    kc = nc.dram_tensor("kc", [128, 8], F32, kind="ExternalInput").ap()
    gum = nc.dram_tensor("gum", [128, 4, S], F32, kind="ExternalInput").ap()
    rb = nc.dram_tensor("rb", [128, 4], U32, kind="ExternalInput").ap()
    msk = nc.dram_tensor("msk", [128, 16], F32, kind="ExternalInput").ap()
    out = nc.dram_tensor("out", [BC * N, H], F32, kind="ExternalOutput").ap()

    with ExitStack() as ctx:
        tc = ctx.enter_context(tile.TileContext(nc))

        consts = ctx.enter_context(tc.tile_pool(name="consts", bufs=1))
        qb_p = ctx.enter_context(tc.tile_pool(name="qb", bufs=1))
        strip_p = ctx.enter_context(tc.tile_pool(name="strip", bufs=2))
        small_p = ctx.enter_context(tc.tile_pool(name="small", bufs=2))
        ps = ctx.enter_context(tc.tile_pool(name="ps", bufs=1, space="PSUM"))

        # ---- constants ----
        ident = consts.tile([128, 128], F32)
        make_identity(nc, ident)
        wkt_sb = consts.tile([128, 8, H], F32)
        nc.sync.dma_start(out=wkt_sb, in_=wkt)
        wq_sb = consts.tile([128, 8, H], F32)
        nc.sync.dma_start(out=wq_sb, in_=wq)
        pet_sb = consts.tile([128, 8, S], F32)
        nc.sync.dma_start(out=pet_sb, in_=pet)
        kc_sb = consts.tile([128, 8], F32)
        nc.sync.dma_start(out=kc_sb, in_=kc)
        gum_sb = consts.tile([128, 4, S], F32)
        nc.sync.dma_start(out=gum_sb, in_=gum)
        rb_sb = consts.tile([128, 4], U32)
        nc.sync.dma_start(out=rb_sb, in_=rb)
        msk_sb = consts.tile([128, 16], F32)
        nc.sync.dma_start(out=msk_sb, in_=msk)


        for g in range(SG):
            r0 = g * ROWS
            qbuf = qb_p.tile([128, 8, QW], F32, tag="qbuf", name=f"qbuf{g}")

            # ---- transpose x into qbuf ----
            for t in range(NT):
                strip = strip_p.tile([128, H], F32, tag="strip",
                                     name=f"strip{g}_{t}")
                nc.sync.dma_start(
                    out=strip, in_=x[r0 + 128 * t : r0 + 128 * t + 128, :]
                )
                for cp in range(2):
                    tp = ps.tile([128, 4, 128], F32, tag="tp", bufs=2,
                                 name=f"tp{g}_{t}_{cp}")
                    for i in range(4):
                        c = 4 * cp + i
                        nc.tensor.matmul(
                            tp[:, i, :],
                            strip[:, 128 * c : 128 * c + 128],
                            ident,
                            is_transpose=True,
                            skip_group_check=True,
                        )
                    nc.scalar.activation(
                        out=qbuf[:, 4 * cp : 4 * cp + 4,
                                 128 * t : 128 * t + 128],
                        in_=tp,
                        func=mybir.ActivationFunctionType.Copy,
                    )

            # ---- xsum via segmented DVE reduces (4 chunks of 8 batches) ----
            xsum = small_p.tile([128, 8, GB], F32, tag="xsum", bufs=1, name=f"xs{g}")
            for q in range(8):
                nc.vector.tensor_reduce(
                    out=xsum[:, :, 4 * q : 4 * q + 4],
                    in_=qbuf[:, :, 400 * q : 400 * q + 400].rearrange(
                        "p c (b s) -> p c b s", s=S
                    ),
                    axis=mybir.AxisListType.X,
                    op=mybir.AluOpType.add,
                )

            # ---- ksumT[na_j, b] = (xsum @ wkt).T + kc, computed directly ----
            kt_ps = ps.tile([128, 8, GB], F32, tag="ktp", bufs=1,
                            name=f"ktp{g}")
            for j in range(8):
                for hc in range(8):
                    nc.tensor.matmul(
                        kt_ps[:, j, :],
                        wkt_sb[:, hc, 128 * j : 128 * j + 128],
                        xsum[:, hc, :],
                        start=(hc == 0),
                        stop=(hc == 7),
                        skip_group_check=True,
                    )
            ksumT = small_p.tile([128, 8, GB], F32, tag="ksumT", bufs=1, name=f"kt{g}")
            nc.vector.tensor_tensor(
                out=ksumT,
                in0=kt_ps,
                in1=kc_sb.to_broadcast([128, 8, GB]),
                op=mybir.AluOpType.add,
            )

            # ---- v[h', c, k, b', n] ----
            v_sb = small_p.tile([128, 8, 2, 16, N], F32, tag="v", bufs=1,
                                name=f"v{g}")
            for ci in range(4):
                vp = ps.tile([128, 2, N, GB], F32, tag="vp", bufs=1,
                             name=f"vp{g}_{ci}")
                for cc in range(2):
                    c = 2 * ci + cc
                    for n in range(N):
                        if TMODE_V:
                            nc.tensor.matmul(
                                vp[:, cc, n, :],
                                wq_sb[:, n, 128 * c : 128 * c + 128],
                                ksumT[:, n, :],
                                is_transpose=True,
                                skip_group_check=True,
                            )
                        else:
                            nc.tensor.matmul(
                                vp[:, cc, n, :],
                                wq_sb[:, n, 128 * c : 128 * c + 128],
                                ksumT[:, n, :],
                                skip_group_check=True,
                            )
                nc.vector.tensor_copy(
                    v_sb[:, 2 * ci : 2 * ci + 2].rearrange(
                        "p c k b n -> p c n (k b)"
                    ),
                    vp,
                )

            # ---- logits + gumbel + argmax + gather, per 16-batch half ----
            for k in range(2):
                yt = 2 * g + k
                y_sb = small_p.tile([128, S], F32, tag="y", name=f"y{yt}")
                # pe part: accumulate over c into [128bn, S]
                lgp = ps.tile([128, 512], F32, tag="lg", bufs=3,
                              name=f"lgp{yt}")
                for c in range(8):
                    nc.tensor.matmul(
                        lgp[:, :S],
                        v_sb[:, c, k].rearrange("p b n -> p (b n)"),
                        pet_sb[:, c, :],
                        start=(c == 0),
                        stop=(c == 7),
                        skip_group_check=True,
                        is_transpose=True if TMODE_LG else None,
                    )
                y16 = small_p.tile([128, 16, S], F32, tag="y16", bufs=1,
                                   name=f"y16_{yt}")
                for m in range(4):
                    lg = ps.tile([128, 512], F32, tag="lg", bufs=3,
                                 name=f"lg{yt}_{m}")
                    col0 = 1600 * k + 400 * m
                    for c in range(8):
                        nc.tensor.matmul(
                            lg[:, :400],
                            v_sb[:, c, k].rearrange("p b n -> p (b n)"),
                            qbuf[:, c, col0 : col0 + 400],
                            start=(c == 0),
                            stop=(c == 7),
                            skip_group_check=True,
                            is_transpose=True if TMODE_LG else None,
                        )
                    # select each row's own batch window via masked multiply
                    for w in range(4):
                        j = 4 * m + w
                        nc.scalar.activation(
                            out=y16[:, j, :],
                            in_=lg[:, 100 * w : 100 * w + 100],
                            func=mybir.ActivationFunctionType.Copy,
                            scale=msk_sb[:, j : j + 1],
                        )
                # y = sum_j y16[:, j, :]  (strided reduce over the 16-axis)
                nc.vector.tensor_reduce(
                    out=y_sb,
                    in_=y16.rearrange("p j s -> p s j"),
                    axis=mybir.AxisListType.X,
                    op=mybir.AluOpType.add,
                )
                # y += pe.v + gumbel
                nc.vector.tensor_tensor(
                    out=y_sb, in0=y_sb, in1=lgp[:, :S], op=mybir.AluOpType.add
                )
                nc.vector.tensor_tensor(
                    out=y_sb, in0=y_sb, in1=gum_sb[:, yt, :],
                    op=mybir.AluOpType.add,
                )
                mx = small_p.tile([128, 8], F32, tag="mx", name=f"mx{yt}")
                idx = small_p.tile([128, 8], U32, tag="idx", name=f"idx{yt}")
                nc.vector.max(mx, y_sb)
                nc.vector.max_index(idx, mx, y_sb)
                gidx = small_p.tile([128, 1], U32, tag="gidx",
                                    name=f"gi{yt}")
                nc.vector.tensor_tensor(
                    out=gidx, in0=idx[:, 0:1], in1=rb_sb[:, yt : yt + 1],
                    op=mybir.AluOpType.add,
                )
                gath = small_p.tile([128, H], F32, tag="gath", bufs=1,
                                    name=f"ga{yt}")
                nc.gpsimd.indirect_dma_start(
                    out=gath[:, :],
                    out_offset=None,
                    in_=x[:, :],
                    in_offset=bass.IndirectOffsetOnAxis(ap=gidx[:, 0:1],
                                                        axis=0),
                )
                nc.sync.dma_start(
                    out=out[128 * yt : 128 * yt + 128, :], in_=gath
                )

    nc.compile()
    return nc


def _host_prep():
    pos = np.arange(S, dtype=np.float32)[:, None]
    div = np.exp(
        np.arange(0, H, 2, dtype=np.float32) * (-math.log(10000.0) / H)
    ).astype(np.float32)
    pe = np.zeros((S, H), dtype=np.float32)
    pe[:, 0::2] = np.sin(pos * div)
    pe[:, 1::2] = np.cos(pos * div)
    pesum = pe.sum(axis=0, dtype=np.float32)
    pet_h = pe.T.reshape(8, 128, S).transpose(1, 0, 2).copy()  # [128p,8hc,S]
    return pe, pesum, pet_h


def _install_profile_shim():
    import sys
    import types

    if "antenv.axon_hooks" not in sys.modules:
        from trn_agent_boot.trn_boot import _ntff_profile_via_ctypes

        hook = _ntff_profile_via_ctypes("/opt/axon/libaxon_pjrt.so")
        mod = types.ModuleType("antenv.axon_hooks")
        mod.get_axon_ntff_profile_hook = lambda: hook
        mod.set_axon_ntff_profile_hook = lambda h: None
        sys.modules["antenv.axon_hooks"] = mod
    import concourse.bass_utils as bu

    bu.upload_artifacts = lambda tmpdir: tmpdir


def kernel(x, Wq, Wk, gumbel, _trace=False):
    global LAST_RESULT
    if _trace:
        _install_profile_shim()
    x = np.ascontiguousarray(np.asarray(x), dtype=np.float32)
    Wq = np.asarray(Wq, dtype=np.float32)
    Wk = np.asarray(Wk, dtype=np.float32)
    gumbel = np.ascontiguousarray(np.asarray(gumbel), dtype=np.float32)

    if "nc" not in _NC_CACHE:
        _NC_CACHE["nc"] = _build_nc()
        _NC_CACHE["prep"] = _host_prep()
    nc = _NC_CACHE["nc"]
    pe, pesum, pet_h = _NC_CACHE["prep"]

    wkt = (Wk.T * SCALE).astype(np.float32)                   # [H, NA]
    kconst = (pesum @ wkt).astype(np.float32)                 # [NA]
    kc_h = kconst.reshape(8, 128).T.copy()                    # [128a, 8n]
    wkt_h = wkt.reshape(8, 128, H).transpose(1, 0, 2).copy()  # [128p,8hc,NA]
    wq_h = Wq.reshape(8, 128, H).transpose(1, 0, 2).copy()    # [128a,8n,H]

    p = np.arange(128)
    gum_r = gumbel.reshape(B, N, S)
    in_maps = []
    for core in range(NCORES):
        b0 = core * BC
        gperm = np.zeros((128, 4, S), dtype=np.float32)
        rbase = np.zeros((128, 4), dtype=np.uint32)
        for yt in range(4):
            bl = 16 * yt + p // 8                              # local batch
            gperm[:, yt, :] = gum_r[b0 + bl, p % 8, :]
            rbase[:, yt] = (bl * S).astype(np.uint32)
        mask2 = (p[:, None] // 8 == np.arange(16)[None, :]).astype(np.float32)
        in_maps.append(
            {
                "x": x[b0 : b0 + BC].reshape(BC * S, H),
                "msk": mask2,
                "wkt": wkt_h,
                "wq": wq_h,
                "pet": pet_h,
                "kc": kc_h,
                "gum": gperm,
                "rb": rbase,
            }
        )

    res = run_bass_kernel_spmd(nc, in_maps, list(range(NCORES)), trace=_trace)
    LAST_RESULT = res

    outp = np.zeros((B, N, H), dtype=np.float32)
    for core in range(NCORES):
        oc = res.results[core]["out"]                          # [512, H]
        for yt in range(4):
            bl = core * BC + 16 * yt + p // 8
            outp[bl, p % 8, :] = oc[128 * yt + p]
    return outp
